# revision 1
# baseline (speedup 1.0000x reference)
"""MultiHeadSelfAttention Trainium2 kernel (8-core SPMD), v2.

Sharding: 16 heads across 8 cores (2 heads each, "h" in {0,1}); every core
computes BOTH batches for its 2 heads. Output rows: core c owns batch c//4,
q-rows (c%4)*512..+512.

Per-core program (fp8 matmuls, f32 accum):
  1. Projections from xT fp8 [D, 2b, L] with fp8 weights scaled x64:
       Q,K: DoubleRow chains -> qt/kt bf16 (x 1/64, ACT-copy drains)
       V:   fp8 chains -> v8 fp8 [128 kpos, 2b, 16 lt, 2h x 65+pad]
            (col 64 of each head block = 1.0: softmax denominator rider;
             v scaled 1/256 so raw po fits fp8/bf16 ranges)
     Chains are emitted strictly before their reader units (Tile deps are
     emission-ordered) and paced via per-quad filler slots.
  2. Attention in 16 units u=(b, j): q-cols j*256. Per unit 8 quads
     (kt pairs): 4 score MMs (bf16, K=64, h-row-tiled concurrent pairs)
     -> sc PSUM [128,1024]; exp: ~56/128 quads on a custom DVE polynomial
     (p(s)^4 ~ lam*e^{s/8}; lam cancels within each head's softmax row mix),
     rest on ScalarE ACT Exp -> ex fp8; AV: DoubleRow MMs accumulate
     po PSUM [65, 2h, 256] -- h0 inline, h1 as a burst after h0 drains
     (a start=True clears the whole PSUM bank's has_written bits, so the
     two h accumulation groups must be sequential in their shared bank).
  3. Per unit: drain po -> SBUF f32 (DVE h0 / ACT h1), compact denominators
     [128, 4] via DMA fold, reciprocal x256 (DVE), stage RAW o + 256/den
     rows into the cc_in shard with gpsimd casting DMAs (f32 -> bf16).
     gpsimd SW-DGE staging is deliberate: HWDGE (sync/scalar) DMA traffic
     concurrent with a collective destabilizes the CC transport.
  4. Two 8-rank AllToAlls (q-halves: even-j units then odd-j units), each
     shipping [8, 130, 256] bf16 shards = all 16 heads' raw o + recip-den
     rows for my q-rows. bf16 payloads: fp8 collectives measured erratic.
  5. Receiver: og_mult normalizes (o * 256/den -> o*64, fp8), then
     out-projection: DoubleRow over 4 peer-pairs with wo fp8 (x64);
     residual: out = psum/4096 + xres (f32); LayerNorm with one batched
     Sqrt table switch at the tail (bn stats per row tile as they finish).
"""

import sys

sys.path.insert(0, "/opt/trn_rl_repo")

import numpy as np
import ml_dtypes

import concourse.bass as bass
import concourse.bacc as bacc
import concourse.tile as tile
from concourse import mybir
from concourse import bass_utils
import bass_rust

BF16 = mybir.dt.bfloat16
F32 = mybir.dt.float32
FP8 = mybir.dt.float8e4
AF = mybir.ActivationFunctionType
DR = mybir.MatmulPerfMode.DoubleRow
MUL = mybir.AluOpType.mult
E4 = ml_dtypes.float8_e4m3

# custom DVE exp: p(s) = ((A3 s + A2) s + A1) s + 1;  p^4 ~ LAM * e^{s/8}
A1 = 0.031379391305728067
A2 = 0.00050919663280734283
A3 = 4.9613446254565463e-06

_PATCHED = False


def _patch_tile_drain():
    """The installed walrus rejects >1 sem wait on a Drain instruction; split
    the TileContext tail-drain waits across multiple drains."""
    global _PATCHED
    if _PATCHED:
        return
    _PATCHED = True

    def _patched(self, tick_clock, wait_clock):
        from concourse.vector_clock import ScopedClock

        probe = self.nc.sync.drain()
        wait_clock.add_sem_waits(
            probe.ins, ScopedClock({None: tick_clock.global_clock})
        )
        si = probe.ins.sync_info
        waits = list(si.on_wait or []) if si is not None else []
        if len(waits) > 1:
            si.on_wait = [waits[0]]
            for w in waits[1:]:
                d2 = self.nc.sync.drain()
                si2 = d2.ins.sync_info
                if si2 is None:
                    d2.ins.sync_info = bass_rust.SyncInfo(on_wait=[w], on_update=[])
                else:
                    si2.on_wait = [w]
        self.nc.all_engine_barrier()
        assert self.sems is not None
        popped = self.nc._tile_sem_poison_stack.pop()
        assert popped is self._sem_poison
        self.nc.clear_and_free_semaphores(list(self.sems.allocated().values()))
        self.nc.all_engine_barrier()

    tile.TileContext._drain_and_barrier = _patched


def _register_exp_op():
    """Register the polynomial-exp custom DVE op (append-only; idempotent)."""
    from concourse import dve_ops
    from concourse.dve_spec import Spec, Src0, C0, C1, C2, One, sq, lower
    from concourse.dve_uop import DveOpSpec
    from concourse.dve_ops import DveOp

    name = "EXP_S8_ANT"
    if name in dve_ops._SUB_OPCODE_FOR_NAME:
        return next(o for o in dve_ops.OPS if o.name == name)

    x = Src0
    body = sq(sq(((C2 * x + C1) * x + C0) * x + One))

    def ref(in0, in1, s0, s1, imm2):
        return ((((imm2 * in0 + s1) * in0 + s0) * in0 + 1.0) ** 2) ** 2

    spec = Spec(body=body, reference=ref)
    row = dve_ops._CUSTOM_DVE_ROW_BASE + len(dve_ops.OPS)
    shas = {}
    for ver in ("v3", "v4"):
        compiled = DveOpSpec(
            name=name, opcode=row, uops=lower(spec, ver=ver), rd1_en=False
        )
        shas[ver] = compiled.sha(ver)
    op = DveOp(name, spec, subdim=False, uops_sha=shas)
    dve_ops.OPS.append(op)
    dve_ops._SUB_OPCODE_FOR_NAME[name] = row
    dve_ops.CUSTOM_DVE_SPECS[name] = spec
    return op


def build_nc(L=2048, D=1024, eps=1e-6, trivial_gamma=False, trivial_beta=False):
    _patch_tile_drain()
    EXP_OP = _register_exp_op()

    KD = D // 128     # 8 contraction tiles over D
    LT = L // 128     # 16 kpos tiles
    NJ = L // 256     # 8 q-chunks of 256 per batch
    QW = 256          # unit q width
    QS = 512          # per-core output rows

    nc = bacc.Bacc(num_devices=8, debug=False)

    xT_d = nc.dram_tensor("xT", [D, 2 * L], FP8, kind="ExternalInput")
    wq_d = nc.dram_tensor("wq", [D, 128], FP8, kind="ExternalInput")
    wk_d = nc.dram_tensor("wk", [D, 128], FP8, kind="ExternalInput")
    wv_d = nc.dram_tensor("wv", [D, 128], FP8, kind="ExternalInput")
    wo_d = nc.dram_tensor("wo", [D, D], FP8, kind="ExternalInput")
    xres_d = nc.dram_tensor("xres", [QS, D], F32, kind="ExternalInput")
    gamma_d = nc.dram_tensor("gamma", [1, D], F32, kind="ExternalInput")
    beta_d = nc.dram_tensor("beta", [1, D], F32, kind="ExternalInput")
    out_d = nc.dram_tensor("out", [QS, D], F32, kind="ExternalOutput")

    with tile.TileContext(nc) as tc:
        with (
            tc.tile_pool(name="singles", bufs=1) as singles,
            tc.tile_pool(name="exp", bufs=10) as expp,
            tc.tile_pool(name="small", bufs=2) as small,
            tc.tile_pool(name="psum", bufs=1, space="PSUM") as psum,
            tc.tile_pool(name="dram", bufs=1, space="DRAM") as dram,
        ):
            # ---------------- loads ----------------
            xT_sb = singles.tile([128, KD, 2, L], FP8)
            wq_sb = singles.tile([128, KD, 128], FP8)
            wk_sb = singles.tile([128, KD, 128], FP8)
            wv_sb = singles.tile([128, KD, 128], FP8)
            for eng, w_sb, w_d in (
                (nc.sync, wk_sb, wk_d),
                (nc.scalar, wq_sb, wq_d),
                (nc.scalar, wv_sb, wv_d),
            ):
                eng.dma_start(
                    out=w_sb, in_=w_d.ap().rearrange("(t p) m -> p t m", p=128)
                )
            # x loads: one DMA per (t, b); batch b=0 first
            xT_r = xT_d.ap().rearrange("(t p) m -> p t m", p=128)
            for b in range(2):
                for lh in range(2):
                    for t in range(KD):
                        nc.sync.dma_start(
                            out=xT_sb[:, t, b, lh * 1024 : (lh + 1) * 1024],
                            in_=xT_r[
                                :, t, b * L + lh * 1024 : b * L + (lh + 1) * 1024
                            ],
                        )
            wo_sb = singles.tile([128, KD, D], FP8)
            xres_sb = singles.tile([128, 4, D], F32)
            gb_sb = singles.tile([128, D], F32)
            bb_sb = singles.tile([128, D], F32)
            with tc.tile_wait_until(0.05):
                wo_r = wo_d.ap().rearrange("(t p) n -> p t n", p=128)
                for t in range(KD):
                    nc.scalar.dma_start(out=wo_sb[:, t, :], in_=wo_r[:, t, :])
                nc.scalar.dma_start(
                    out=xres_sb,
                    in_=xres_d.ap().rearrange("(t p) d -> p t d", p=128),
                )
                if not trivial_gamma:
                    nc.scalar.dma_start(
                        out=gb_sb,
                        in_=bass.AP(tensor=gamma_d, offset=0, ap=[[0, 128], [1, D]]),
                    )
                if not trivial_beta:
                    nc.scalar.dma_start(
                        out=bb_sb,
                        in_=bass.AP(tensor=beta_d, offset=0, ap=[[0, 128], [1, D]]),
                    )
            eps_sb = singles.tile([128, 1], F32)
            nc.vector.memset(eps_sb, eps)

            # ---------------- projection chains ----------------
            qt_sb = singles.tile([128, 2, L], BF16)
            kt_sb = singles.tile([128, 2, L], BF16)
            v8_sb = singles.tile([128, 2, LT, 144], FP8)
            nc.vector.memset(v8_sb, 1.0)  # head-col 64/129 stay exactly 1.0

            pj_tile = psum.tile([128, 2, 256], F32, tag="pj", bufs=1, name="pj")

            def chain_qk(w_sb, o_sb, b, lc2):
                # two 256-col accumulation groups into the pj halves, then one
                # paired ACT drain (cast bf16, x1/64)
                for half in range(2):
                    sl = slice((2 * lc2 + half) * 256, (2 * lc2 + half + 1) * 256)
                    for i in range(KD // 2):
                        nc.tensor.matmul(
                            pj_tile[:, half, :],
                            lhsT=w_sb[:, 2 * i : 2 * i + 2, :],
                            rhs=xT_sb[:, 2 * i : 2 * i + 2, b, sl],
                            start=(i == 0),
                            stop=(i == KD // 2 - 1),
                            perf_mode=DR,
                        )
                nc.scalar.activation(
                    out=o_sb[:, b, 2 * lc2 * 256 : (2 * lc2 + 2) * 256],
                    in_=pj_tile,
                    func=AF.Copy,
                    scale=1.0 / 64,
                )

            def chain_v(b, lt2):
                # lt tiles 2*lt2, 2*lt2+1 into pj halves cols 0:128
                for half in range(2):
                    lt = 2 * lt2 + half
                    for kd in range(KD):
                        nc.tensor.matmul(
                            pj_tile[:, half, 0:128],
                            lhsT=xT_sb[:, kd, b, lt * 128 : (lt + 1) * 128],
                            rhs=wv_sb[:, kd, :],
                            start=(kd == 0),
                            stop=(kd == KD - 1),
                        )
                nc.scalar.activation(
                    out=v8_sb[:, b, 2 * lt2 : 2 * lt2 + 2, 0:130].rearrange(
                        "p t (h a) -> p t h a", h=2
                    )[:, :, :, 0:64],
                    in_=pj_tile[:, :, 0:128].rearrange(
                        "p t (h a) -> p t h a", h=2
                    ),
                    func=AF.Copy,
                    scale=1.0 / 256,
                )

            # ---------------- attention units ----------------
            cc_in = [dram.tile([8, 130, QW], BF16, name=f"cci{h}") for h in range(2)]
            cc_out = [dram.tile([8 * 130, QW], BF16, name=f"cco{h}") for h in range(2)]
            og_sb = [singles.tile([128, 8, QW], BF16, name=f"og{h}") for h in range(2)]
            S_sb = [singles.tile([128, 8, QW], BF16, name=f"S{h}") for h in range(2)]
            og_n = [singles.tile([128, 8, QW], FP8, name=f"ogn{h}") for h in range(2)]

            def attn_unit(b, j, fillers=()):
                # po h0/h1 share one PSUM bank: a start=True clears the WHOLE
                # bank's has_written bits, so the two h accumulation groups
                # must be fully sequential, with h0 drained before h1 starts.
                po = psum.tile([65, 2, QW], F32, tag="po", bufs=1,
                               name=f"po_{b}_{j}")
                po_s = small.tile([65, 2, QW], F32, tag="pos",
                                  name=f"pos_{b}_{j}")
                qsl = slice(j * QW, (j + 1) * QW)
                exs = []
                for t in range(LT // 2):
                    sc = psum.tile([128, 1024], F32, tag="sc", bufs=3,
                                   name=f"sc_{b}_{j}_{t}")
                    # layout: h0 cols 0:512 (kt pair), h1 cols 512:1024
                    for kk in range(2):
                        for h in range(2):
                            col = h * 512 + kk * 256
                            nc.tensor.matmul(
                                sc[:, col : col + 256],
                                lhsT=kt_sb[
                                    64 * h : 64 * h + 64,
                                    b,
                                    (2 * t + kk) * 128 : (2 * t + kk + 1) * 128,
                                ],
                                rhs=qt_sb[64 * h : 64 * h + 64, b, qsl],
                                start=True,
                                stop=True,
                            )
                    ex = expp.tile([128, 1024], FP8, tag="ex",
                                   name=f"ex_{b}_{j}_{t}")
                    if t % 2 == 0 or (t == 7 and b == 1):
                        nc.scalar.activation(
                            out=ex, in_=sc, func=AF.Exp, scale=0.125
                        )
                    else:
                        nc.vector._custom_dve(
                            EXP_OP, out=ex, in0=sc, s0=A1, s1=A2, imm2=A3
                        )
                    exs.append(ex)
                    nc.tensor.matmul(
                        po[:, 0, :],
                        lhsT=v8_sb[:, b, 2 * t : 2 * t + 2, 0:65],
                        rhs=ex[:, 0:512].rearrange("p (i n) -> p i n", i=2),
                        start=(t == 0),
                        stop=(t == LT // 2 - 1),
                        perf_mode=DR,
                    )
                    if t < len(fillers):
                        for fn, args in fillers[t]:
                            fn(*args)
                nc.vector.tensor_copy(out=po_s[:, 0, :], in_=po[:, 0, :])
                for t in range(LT // 2):
                    nc.tensor.matmul(
                        po[:, 1, :],
                        lhsT=v8_sb[:, b, 2 * t : 2 * t + 2, 65:130],
                        rhs=exs[t][:, 512:1024].rearrange(
                            "p (i n) -> p i n", i=2
                        ),
                        start=(t == 0),
                        stop=(t == LT // 2 - 1),
                        perf_mode=DR,
                    )
                nc.scalar.activation(
                    out=po_s[:, 1, :], in_=po[:, 1, :], func=AF.Copy
                )
                # compact reciprocal denominators -> [128, 4] (= 64/den)
                rc4 = small.tile([128, 4], F32, tag="rc4", name=f"rc4_{b}_{j}")
                nc.sync.dma_start(out=rc4, in_=po_s[64:65, :, :])
                nc.vector.reciprocal(out=rc4, in_=rc4)
                nc.vector.tensor_scalar_mul(out=rc4, in0=rc4, scalar1=256.0)
                # ship raw o (fp8 cast in the DMA) + the 64/den rows;
                # the receiver normalizes.
                half, peer = j % 2, b * 4 + j // 2
                for h in range(2):
                    nc.gpsimd.dma_start(
                        out=cc_in[half][peer, 64 * h : 64 * h + 64, :],
                        in_=po_s[0:64, h, :],
                    )
                nc.gpsimd.dma_start(
                    out=cc_in[half][peer, 128:130, :], in_=rc4
                )

            # Chains MUST be emitted before any unit that reads their output
            # (Tile deps are emission-ordered). Pace them via per-quad filler
            # slots so the exp stream starts after only 3 chains.
            K_ = lambda b, i: (chain_qk, (wk_sb, kt_sb, b, i))
            Q_ = lambda b, i: (chain_qk, (wq_sb, qt_sb, b, i))
            V_ = lambda b, i: (chain_v, (b, i))

            units = [(0, j) for j in (0, 2, 4, 6)] + \
                    [(1, j) for j in (0, 2, 4, 6)] + \
                    [(0, j) for j in (1, 3, 5, 7)] + \
                    [(1, j) for j in (1, 3, 5, 7)]
            fillmap = {
                (0, 0): [[V_(0, 1), K_(0, 1)], [V_(0, 2)], [V_(0, 3), K_(0, 2)],
                         [V_(0, 4)], [V_(0, 5), K_(0, 3)], [V_(0, 6)],
                         [V_(0, 7)], [Q_(0, 1)]],
                (0, 2): [[K_(1, 0)], [Q_(1, 0)], [V_(1, 0)], [V_(1, 1)],
                         [K_(1, 1)], [V_(1, 2)], [V_(1, 3)], [Q_(0, 2)]],
                (0, 4): [[K_(1, 2)], [V_(1, 4)], [V_(1, 5)], [K_(1, 3)],
                         [V_(1, 6)], [V_(1, 7)], [Q_(0, 3)]],
                (0, 6): [[Q_(1, 1)], [Q_(1, 2)], [Q_(1, 3)]],
            }
            for fn, args in (K_(0, 0), Q_(0, 0), V_(0, 0)):
                fn(*args)
            for ui, (b, j) in enumerate(units):
                attn_unit(b, j, fillmap.get((b, j), ()))
                if ui == 7:
                    nc.gpsimd.collective_compute(
                        "AllToAll",
                        mybir.AluOpType.bypass,
                        replica_groups=[[0, 1, 2, 3, 4, 5, 6, 7]],
                        ins=[cc_in[0].opt()],
                        outs=[cc_out[0].opt()],
                    )
            nc.gpsimd.collective_compute(
                "AllToAll",
                mybir.AluOpType.bypass,
                replica_groups=[[0, 1, 2, 3, 4, 5, 6, 7]],
                ins=[cc_in[1].opt()],
                outs=[cc_out[1].opt()],
            )

            def recv_og(half, engine):
                engine.dma_start(
                    out=og_sb[half],
                    in_=bass.AP(
                        tensor=cc_out[half].tensor,
                        offset=0,
                        ap=[[QW, 128], [130 * QW, 8], [1, QW]],
                    ),
                )
                # S[p, peer, q] = (256/den)[h=p//64, q] of peer's shard
                for h2 in range(2):
                    engine.dma_start(
                        out=S_sb[half][64 * h2 : 64 * h2 + 64, :, :],
                        in_=bass.AP(
                            tensor=cc_out[half].tensor,
                            offset=(128 + h2) * QW,
                            ap=[[0, 64], [130 * QW, 8], [1, QW]],
                        ),
                    )
                for h2 in range(2):
                    nc.vector.tensor_mul(
                        out=og_n[half][:, 4 * h2 : 4 * h2 + 4, :],
                        in0=og_sb[half][:, 4 * h2 : 4 * h2 + 4, :],
                        in1=S_sb[half][:, 4 * h2 : 4 * h2 + 4, :],
                    )

            # ---------------- out-projection + residual + LN ----------------
            BN_STATS_DIM = nc.vector.BN_STATS_DIM
            BN_AGGR_DIM = nc.vector.BN_AGGR_DIM
            out_acc = [
                small.tile([128, D], F32, tag="oac", bufs=4, name=f"oac{qt}")
                for qt in range(4)
            ]

            def outproj_qt(half, qt):
                # qt in 0..3 global; local row tile within half: qt%2
                for dmt in range(2):
                    ps = psum.tile([128, 1024], F32, tag="sc", bufs=3,
                                   name=f"op_{qt}_{dmt}")
                    for p in range(4):
                        nc.tensor.matmul(
                            ps[:, 0:512],
                            lhsT=og_n[half][
                                :, 2 * p : 2 * p + 2,
                                (qt % 2) * 128 : (qt % 2) * 128 + 128,
                            ],
                            rhs=wo_sb[:, 2 * p : 2 * p + 2,
                                      dmt * 512 : (dmt + 1) * 512],
                            start=(p == 0),
                            stop=(p == 3),
                            perf_mode=DR,
                        )
                    dsl = slice(dmt * 512, (dmt + 1) * 512)
                    nc.vector.scalar_tensor_tensor(
                        out=out_acc[qt][:, dsl],
                        in0=ps[:, 0:512],
                        scalar=1.0 / 4096,
                        in1=xres_sb[:, qt, dsl],
                        op0=MUL,
                        op1=mybir.AluOpType.add,
                    )

            mvs = [
                small.tile([128, BN_AGGR_DIM], F32, tag="mv", bufs=4,
                           name=f"mv{qt}")
                for qt in range(4)
            ]

            def ln_stats_qt(qt):
                o = out_acc[qt]
                stats = small.tile([128, 2, BN_STATS_DIM], F32, tag="stats")
                for s in range(2):
                    nc.vector.bn_stats(
                        out=stats[:, s, :], in_=o[:, s * 512 : (s + 1) * 512]
                    )
                nc.vector.bn_aggr(out=mvs[qt], in_=stats)

            def ln_finish():
                # one Sqrt table switch for all four row tiles
                var4 = small.tile([128, 4], F32, tag="var4")
                for qt in range(4):
                    nc.vector.tensor_copy(
                        out=var4[:, qt : qt + 1], in_=mvs[qt][:, 1:2]
                    )
                std4 = small.tile([128, 4], F32, tag="std4")
                nc.scalar.activation(
                    out=std4, in_=var4, func=AF.Sqrt, bias=eps_sb, scale=1.0
                )
                nc.vector.reciprocal(out=std4, in_=std4)
                for qt in range(4):
                    o = out_acc[qt]
                    nc.vector.tensor_scalar(
                        out=o, in0=o,
                        scalar1=mvs[qt][:, 0:1], scalar2=std4[:, qt : qt + 1],
                        op0=mybir.AluOpType.subtract, op1=MUL,
                    )
                    if not trivial_gamma:
                        nc.vector.tensor_mul(out=o, in0=o, in1=gb_sb)
                    if not trivial_beta:
                        nc.vector.tensor_add(out=o, in0=o, in1=bb_sb)
                    nc.sync.dma_start(
                        out=out_d.ap().rearrange("(t p) d -> p t d", p=128)[
                            :, qt, :
                        ],
                        in_=o,
                    )

            with tc.tile_wait_until(1.0):
                recv_og(0, nc.gpsimd)
                for qt in (0, 1):
                    outproj_qt(0, qt)
                    ln_stats_qt(qt)
            with tc.tile_wait_until(1.1):
                recv_og(1, nc.gpsimd)
                for qt in (2, 3):
                    outproj_qt(1, qt)
                    ln_stats_qt(qt)
                ln_finish()
    nc.compile()
    return nc


def make_in_maps(x, Wq, Wk, Wv, Wo, ln_gamma, ln_beta, L, D):
    B = x.shape[0]
    QS = 512
    xT8 = np.ascontiguousarray(
        x.transpose(2, 0, 1).reshape(D, B * L)
    )
    xT8 = np.clip(xT8, -240, 240).astype(E4)
    wo8 = np.clip(Wo * 64.0, -240, 240).astype(E4)
    in_maps = []
    for c in range(8):
        cols = slice(c * 128, (c + 1) * 128)
        bc, qs = c // 4, c % 4
        in_maps.append(
            {
                "xT": xT8,
                "wq": np.clip(
                    np.ascontiguousarray(Wq[:, cols]) * 64.0, -240, 240
                ).astype(E4),
                "wk": np.clip(
                    np.ascontiguousarray(Wk[:, cols]) * 64.0, -240, 240
                ).astype(E4),
                "wv": np.clip(
                    np.ascontiguousarray(Wv[:, cols]) * 64.0, -240, 240
                ).astype(E4),
                "wo": wo8,
                "xres": np.ascontiguousarray(
                    x[bc, qs * QS : (qs + 1) * QS]
                ).astype(np.float32),
                "gamma": np.ascontiguousarray(ln_gamma[None, :]).astype(np.float32),
                "beta": np.ascontiguousarray(ln_beta[None, :]).astype(np.float32),
            }
        )
    return in_maps


def assemble(results, L, D):
    QS = 512
    out = np.zeros((2, L, D), np.float32)
    for c in range(8):
        bc, qs = c // 4, c % 4
        out[bc, qs * QS : (qs + 1) * QS] = results[c]["out"]
    return out


def run(x, Wq, Wk, Wv, Wo, ln_gamma, ln_beta, trace=False):
    B, L, D = x.shape
    nc = build_nc(
        L=L, D=D,
        trivial_gamma=bool(np.all(ln_gamma == 1.0)),
        trivial_beta=bool(np.all(ln_beta == 0.0)),
    )
    in_maps = make_in_maps(x, Wq, Wk, Wv, Wo, ln_gamma, ln_beta, L, D)
    res = bass_utils.run_bass_kernel_spmd(
        nc, in_maps, core_ids=list(range(8)), trace=trace
    )
    return assemble(res.results, L, D), res


def kernel(x, Wq, Wk, Wv, Wo, ln_gamma, ln_beta):
    out, _ = run(
        np.asarray(x, np.float32),
        np.asarray(Wq, np.float32),
        np.asarray(Wk, np.float32),
        np.asarray(Wv, np.float32),
        np.asarray(Wo, np.float32),
        np.asarray(ln_gamma, np.float32),
        np.asarray(ln_beta, np.float32),
    )
    return out

